# revision 74
# baseline (speedup 1.0000x reference)
"""TRN2 Bass kernel for nn_NMS (offset min-sum LDPC decoder, batch 256).

Self-contained: derives all index tables from the H input at call time,
shards the batch across 8 NeuronCores (32 per core), runs one SPMD Bass
program via run_bass_kernel_spmd, and gathers the full [256, 576] output.

Per-core layout: two independent 16-batch streams, each using all 128
partitions as 8 row-blocks x 16 batch; a stream's edges live on the free
axis as [18 rows x 16 slots] = 288 fp16 values.

The check-node update computes each edge's leave-one-out |X| minimum
directly with an exclusive-min tournament: a half-split min tree
(u1/u2/u3) plus sibling-subtree min combines, all packed 2x DVE ops.
Position-based exclusion reproduces the reference's argmin/tie
semantics exactly at full fp16 precision -- no mantissa truncation,
no argmin mask, no select.

Pipeline (per iteration): the two streams' DVE chains run back-to-back
(tiny dummy-copy serializers pin the A0,B0,A1,..,B2 order; the greedy
Tile list scheduler would otherwise interleave them) while the colsum
phases (Pool strip gathers -> PE one-hot matmuls -> ACT PSUM
evacuation -> Pool zrep re-expansion gather) of the previous stream
run concurrently on the other engines.

Iteration 0 is fed by host-precomputed tb0 = |x0| and sg0 = +-0.5
(pointwise transforms of the iteration-0 edge expansion of r), so its
chain starts at the min tree as soon as one DMA lands.
"""
import numpy as np
from contextlib import ExitStack

import concourse.bass as bass
import concourse.tile as tile
from concourse import mybir

FP32 = mybir.dt.float32
FP16 = mybir.dt.float16
I32 = mybir.dt.int32
U16 = mybir.dt.uint16
ALU = mybir.AluOpType
AX = mybir.AxisListType
AF = mybir.ActivationFunctionType

P = 128
NSTR = 2         # independent batch streams per core
B = 16           # batch per stream
BC = NSTR * B    # batch per core
NBLK = 8
RPB = 18         # rows per block
KPAD = 16        # padded row degree (power of 2 for the exclusive-min tree)
ROW_DEG = 15
EPB = RPB * KPAD  # 288 edge slots per block per stream
EPAD = EPB       # gather output width
N = 576          # columns
HALF = N // 2
D_KEEP = 2       # strip slots per (block, col)
ITERS = 3
BIG = np.float16(30000.0)
ZSLOT = EPB           # zero slot in E_ext

STRIP_WAITS = False  # sems are load-bearing on HW: stripping same-engine
STRIP_ENGINES = ()  # waits produced NaN results on the real device
PHASE_LOG = {}  # (it, s) -> name of the chain's last DVE instruction
RBASE = EPB + 1       # per-block r region base
RW = 104              # per-block r region width
AUXB = RBASE + RW     # aux slots base
NAUX = 32
EXTW = AUXB + NAUX    # max gather window width


# ---------------------------------------------------------------- tables ----
def build_tables(H):
    MROWS = H.shape[0]
    cols = np.array([np.nonzero(H[m])[0] for m in range(MROWS)], dtype=np.int64)
    assert cols.shape == (MROWS, ROW_DEG)
    coldeg = H.sum(0)
    heat = np.array([-coldeg[cols[m]].max() for m in range(MROWS)])
    order = list(np.argsort(heat, kind="stable"))
    blocks = [[] for _ in range(NBLK)]
    cnt = np.zeros((NBLK, N), dtype=np.int32)
    for m in order:
        best, bestkey = None, None
        for j in range(NBLK):
            if len(blocks[j]) >= RPB:
                continue
            key = tuple(np.sort(cnt[j, cols[m]])[::-1])
            if best is None or key < bestkey:
                best, bestkey = j, key
        blocks[best].append(m)
        cnt[best, cols[m]] += 1
    assign = np.zeros(MROWS, dtype=np.int64)
    for j, b in enumerate(blocks):
        for m in b:
            assign[m] = j

    def cost_vec(c):
        h = np.bincount(c.flatten(), minlength=8)
        return (int(h[4:].sum()), int(h[3]), int(h[2]))

    for _ in range(12):
        improved = False
        for m1 in range(MROWS):
            for m2 in range(m1 + 1, MROWS):
                j1, j2 = assign[m1], assign[m2]
                if j1 == j2:
                    continue
                c = cnt.copy()
                c[j1, cols[m1]] -= 1
                c[j1, cols[m2]] += 1
                c[j2, cols[m2]] -= 1
                c[j2, cols[m1]] += 1
                if cost_vec(c) < cost_vec(cnt):
                    assign[m1], assign[m2] = j2, j1
                    cnt = c
                    improved = True
        if not improved or cnt.max() <= D_KEEP + 1:
            break
    assert cnt.max() <= D_KEEP + 1, f"strip depth {cnt.max()} > {D_KEEP + 1}"
    rows_of_block = [np.array([m for m in range(MROWS) if assign[m] == j],
                              dtype=np.int64) for j in range(NBLK)]

    colidx = np.full((NBLK, RPB, KPAD), N, dtype=np.int64)
    for j in range(NBLK):
        for mm, m in enumerate(rows_of_block[j]):
            colidx[j, mm, :ROW_DEG] = cols[m]

    pos_lists = [[[] for _ in range(N)] for _ in range(NBLK)]
    for j in range(NBLK):
        for mm in range(RPB):
            for k in range(ROW_DEG):
                n = colidx[j, mm, k]
                pos_lists[j][n].append(mm * KPAD + k)

    # r-slot assignment: each col's r lives in one block's strips (compact
    # per-block r region keeps the gather window small)
    r_block = np.full(N, -1, dtype=np.int64)
    rfull = []
    rcols = [[] for _ in range(NBLK)]
    r_rank = np.full(N, -1, dtype=np.int64)
    for n in range(N):
        cand = [j for j in range(NBLK) if len(pos_lists[j][n]) < D_KEEP]
        if cand:
            j = min(cand, key=lambda j: (len(rcols[j]), len(pos_lists[j][n])))
            r_block[n] = j
            r_rank[n] = len(rcols[j])
            rcols[j].append(int(n))
        else:
            rfull.append(int(n))
    assert len(rfull) <= 16, f"too many rfull cols: {len(rfull)}"

    # strip slots; overflow entries become aux slots (pre-summed pairs)
    slots = np.full((NBLK, N, D_KEEP), ZSLOT, dtype=np.int64)
    aux = []
    for j in range(NBLK):
        for n in range(N):
            lst = list(pos_lists[j][n])
            if r_block[n] == j:
                lst.append(RBASE + int(r_rank[n]))
            assert len(lst) <= D_KEEP + 1, f"depth {len(lst)} at {(j, n)}"
            if len(lst) > D_KEEP:
                q, extra = lst[D_KEEP - 1], lst[D_KEEP]
                lst = lst[:D_KEEP - 1] + [AUXB + len(aux)]
                aux.append((int(q), int(extra)))
            for d, v in enumerate(lst):
                slots[j, n, d] = v
    for n in rfull:
        for j in range(NBLK):
            if slots[j, n, D_KEEP - 1] < EPB and len(rcols[j]) < RW:
                q = int(slots[j, n, D_KEEP - 1])
                slots[j, n, D_KEEP - 1] = AUXB + len(aux)
                aux.append((q, RBASE + len(rcols[j])))
                rcols[j].append(int(n))
                break
        else:
            raise AssertionError(f"no aux-able slot for rfull col {n}")
    assert len(aux) <= NAUX, f"too many aux slots: {len(aux)}"
    assert max(len(c) for c in rcols) <= RW, \
        f"r region overflow: {[len(c) for c in rcols]}"

    # ---- wrapped uint16 index tensors (one 16-partition group per block) ----
    def wrap(vals_per_block, num_idxs):
        t = np.zeros((P, num_idxs // 16), dtype=np.uint16)
        for c in range(8):
            v = vals_per_block[c]
            for i in range(num_idxs):
                t[16 * c + i % 16, i // 16] = v[i]
        return t

    zvals = [colidx[j].reshape(-1) for j in range(NBLK)]           # 288 each
    svals = []
    for j in range(NBLK):
        v = np.empty(N * D_KEEP, dtype=np.int64)
        i = 0
        for h in range(2):
            for d in range(D_KEEP):
                for c in range(HALF):
                    v[i] = slots[j, h * HALF + c, d]
                    i += 1
        svals.append(v)
    zidx = wrap(zvals, EPB)
    sidx = wrap(svals, N * D_KEEP)

    # one-hot cross-block sum + replicate: W[(j',b'), (j,b)] = (b'==b)
    wmat = np.zeros((P, P), dtype=np.float16)
    for jp in range(NBLK):
        for bp in range(B):
            for j in range(NBLK):
                wmat[jp * B + bp, j * B + bp] = 1.0
    return dict(zidx=zidx, sidx=sidx, wmat=wmat, colidx=colidx,
                aux=aux, rcols=rcols)


def build_x0(r_stream, colidx):
    """Iteration-0 gather: x0[(j,b), (mm,k)] = r[b, col]."""
    rh = r_stream.astype(np.float16)
    rpad = np.concatenate([rh, np.full((B, 1), BIG, np.float16)], axis=1)
    x0 = rpad[:, colidx]                      # [B, NBLK, RPB, KPAD]
    x0 = x0.transpose(1, 0, 2, 3).reshape(P, EPB)
    return np.ascontiguousarray(x0)


def build_ts0(r_stream, colidx):
    """Host iteration-0 staging: [tb0 | sg0] packed [P, 2, EPB] fp16 --
    |x0| and +-0.5 sign of x0 (exclusion is positional, no tie-break)."""
    x0 = build_x0(r_stream, colidx)
    tb0 = (x0.view(np.uint16) & np.uint16(0x7FFF)).view(np.float16)
    sg0 = np.where(x0 >= 0, np.float16(0.5), np.float16(-0.5))
    return np.ascontiguousarray(np.stack([tb0, sg0], axis=1))


def build_r4(r, rcols):
    """Zero slot + per-block r region of E_ext for both streams:
    out[(j,b), s, 1+i] = r[s*B + b, rcols[j][i]]."""
    out = np.zeros((P, NSTR, 1 + RW), np.float16)
    for s in range(NSTR):
        rh = r[s * B:(s + 1) * B].astype(np.float16)
        for j in range(NBLK):
            cs = rcols[j]
            out[j * B:(j + 1) * B, s, 1:1 + len(cs)] = rh[:, cs]
    return np.ascontiguousarray(out)


def build_stat(tables):
    """Packed static table [P, 218] u16: sidx | zidx | wmat(fp16)."""
    out = np.zeros((P, 218), np.uint16)
    out[:, 0:72] = tables["sidx"]
    out[:, 72:90] = tables["zidx"]
    out[:, 90:218] = tables["wmat"].view(np.uint16)
    return np.ascontiguousarray(out)


# ---------------------------------------------------------------- kernel ----
def strip_same_engine_waits(nc):
    """Remove semaphore waits that are trivially satisfied by program order.

    Engines execute their instruction stream in order, so a RAW/WAW/WAR
    dependency whose producer runs earlier on the SAME engine needs no
    semaphore: by the time the consumer issues, the producer has finished.
    Tile emits these waits anyway; each costs ~95ns of sequencer stall
    (sem propagation + re-decode) between back-to-back dependent ops.

    Conservative rules: only strip 'sem-ge-imm' waits on semaphores that
    are (a) never decremented or register-driven anywhere, and (b) already
    incremented to >= wait_value by earlier same-engine instructions that
    complete synchronously (DMA-issuing instructions complete async, so
    they never count).
    """
    def dmaish(tname):
        return "DMA" in tname or "TriggerDma" in tname or "Remote" in tname
    insts = []
    for f in nc.m.functions:
        for blk in f.blocks:
            insts.extend(blk.instructions)
    bad = set()
    updaters = {}
    for pos, i in enumerate(insts):
        si = i.sync_info
        if si is None:
            continue
        for u in si.on_update:
            if u.sync_type != "semaphore":
                continue
            if u.update_mode == "sem-inc" and u.update_reg is None:
                updaters.setdefault(u.id, []).append(
                    (pos, i.engine, not dmaish(type(i).__name__),
                     u.update_value))
            else:
                bad.add(u.id)
    for pos, i in enumerate(insts):
        si = i.sync_info
        if si is None or not si.on_wait:
            continue
        keep = []
        changed = False
        for w in si.on_wait:
            strip = False
            if (w.sync_type == "semaphore"
                    and w.wait_mode == "sem-ge-imm"
                    and w.wait_reg is None and w.id not in bad):
                cnt = 0
                for upos, ueng, usync, uval in updaters.get(w.id, ()):
                    if upos < pos and ueng == i.engine and usync:
                        cnt += uval
                if cnt >= w.wait_value and str(i.engine) in STRIP_ENGINES:
                    strip = True
            if strip:
                changed = True
            else:
                keep.append(w)
        if changed:
            i.sync_info = mybir.SyncInfo(
                on_wait=keep, on_update=list(si.on_update))


def hoist_waits(nc, max_embedded=1):
    """Split multi-wait instructions into standalone EventSemaphore waits.

    The walrus build used by the axon compile path only supports a single
    sync-wait slot on most TPB instruction structs; Tile attaches one wait
    per producer proc.  Hoist the extras onto the instruction's sequencer
    as separate wait instructions (exactly what raw-bass wait_ge emits).
    """
    k = 0
    for f in nc.m.functions:
        for b in f.blocks:
            insts = b.instructions
            out = []
            for i in insts:
                tname = type(i).__name__
                si = i.sync_info
                if (si is not None and tname != "InstEventSemaphore"
                        and len(si.on_wait) > max_embedded):
                    waits = list(si.on_wait)
                    keep = waits[:max_embedded]
                    for w in waits[max_embedded:]:
                        es = mybir.InstEventSemaphore(
                            name=f"hoistw{k}", ins=[], outs=[])
                        k += 1
                        es.engine = i.engine
                        es.sync_info = mybir.SyncInfo(on_wait=[w], on_update=[])
                        nc.inst_map[es.name] = es
                        out.append(es)
                    i.sync_info = mybir.SyncInfo(
                        on_wait=keep, on_update=list(si.on_update))
                out.append(i)
            b.instructions = out


def build_bass(alpha, beta, aux=()):
    """alpha/beta: lists of 3 floats (baked as immediates).
    aux: [(q, extra)] E_ext position pairs pre-summed into aux slots."""
    nc = bass.Bass("TRN2", target_bir_lowering=False, debug=False)
    ts0_d = [nc.dram_tensor(f"ts0_{s}", [P, 2 * EPB], FP16, kind="ExternalInput")
             for s in range(NSTR)]
    r4_d = nc.dram_tensor("r4", [P, NSTR * (1 + RW)], FP16, kind="ExternalInput")
    stat_d = nc.dram_tensor("stat", [P, 218], U16, kind="ExternalInput")
    out_d = nc.dram_tensor("out", [BC, N], FP32, kind="ExternalOutput")

    # effective strip-gather window (aux region only if used)
    extw = AUXB + len(aux) if aux else RBASE + RW

    with tile.TileContext(nc) as tc:
        with ExitStack() as ctx:
            pool = ctx.enter_context(tc.tile_pool(name="main", bufs=1))
            pspool = ctx.enter_context(tc.tile_pool(name="ps", bufs=1, space="PSUM"))

            def stile(shape, dtype, name):
                return [pool.tile(shape, dtype, name=f"{name}{s}")
                        for s in range(NSTR)]

            E2 = pool.tile([P, NSTR, EXTW + 1], FP16, name="E2")
            E = [E2[:, s] for s in range(NSTR)]
            zrep = stile([P, N + 2], FP16, "zrep")
            Xg = stile([P, EPAD], FP16, "Xg")
            X = stile([P, RPB, KPAD], FP16, "X")
            tb = stile([P, RPB, KPAD], FP16, "tb")
            sgn = stile([P, RPB, KPAD], FP16, "sgn")
            sp = stile([P, RPB, KPAD], FP16, "sp")
            u1 = stile([P, RPB, 8], FP16, "u1")
            u2 = stile([P, RPB, 4], FP16, "u2")
            u3 = stile([P, RPB, 2], FP16, "u3")
            ma = stile([P, EPB], FP16, "ma")
            mb = stile([P, EPB], FP16, "mb")
            em = stile([P, EPB], FP16, "em")
            wE = stile([P, EPB], FP16, "wE")
            prod = stile([P, RPB], FP32, "prod")
            pr = stile([P, RPB], FP16, "pr")
            G = stile([P, 2, D_KEEP, HALF], FP16, "G")
            zout2 = pool.tile([P, NSTR, N], FP32, name="zout2")
            zout = [zout2[:, s] for s in range(NSTR)]
            ts0 = stile([P, 2, RPB, KPAD], FP16, "ts0")
            tb0 = [ts0[s][:, 0] for s in range(NSTR)]
            sg0 = [ts0[s][:, 1] for s in range(NSTR)]
            zpsA = [pspool.tile([P, HALF], FP32, name=f"zpsA{s}")
                    for s in range(NSTR)]
            zpsB = [pspool.tile([P, HALF], FP32, name=f"zpsB{s}")
                    for s in range(NSTR)]
            stat = pool.tile([P, 218], U16, name="stat")
            sidx = stat[:, 0:72]
            zidx = stat[:, 72:90]
            wmat = stat[:, 90:218].bitcast(FP16)

            # ---- static loads: 4 batched DMAs (HWDGE is one serialized
            # resource at ~630ns fixed cost per transfer), deadline order.
            nc.sync.dma_start(ts0[0][:].rearrange("p a b c -> p (a b c)"),
                              ts0_d[0][:])
            nc.scalar.dma_start(ts0[1][:].rearrange("p a b c -> p (a b c)"),
                                ts0_d[1][:])
            nc.sync.dma_start(stat[:], stat_d[:])
            nc.scalar.dma_start(
                E2[:, :, ZSLOT:ZSLOT + 1 + RW],
                r4_d[:].rearrange("p (a b) -> p a b", a=NSTR))
            for s in range(NSTR):
                nc.vector.memset(zrep[s][:, N:N + 2], float(BIG))
                nc.vector.memset(
                    E[s][:, 0:EPB].rearrange("p (a b) -> p a b",
                                             a=RPB)[:, :, ROW_DEG:KPAD], 0.0)
            # PE p-state warm-up nudge
            nc.tensor.matmul(zpsA[0][0:1, 0:1], lhsT=wmat[0:B, 0:1],
                             rhs=wmat[0:B, 0:1], start=True, stop=True)

            def chain(s, it):
                """Min-sum row phase for (stream s, iteration it): consumes
                Xg (or tb0/sg0 at it==0) and E, produces E in edge layout.

                The first op(s) are dummy [P,1] copies that read the PREVIOUS
                chain's E tile and write a cell this chain overwrites anyway:
                a pure scheduling dependency that serializes the six chains
                A0,B0,A1,..,B2 on DVE (the greedy list scheduler would
                otherwise interleave both streams' chains, making both E
                tensors late and killing the chain/colsum overlap).  The
                same-engine wait strip makes these hops free at runtime."""
                al = float(alpha[it])
                Eedge = E[s][:, 0:EPB].rearrange("p (a b) -> p a b", a=RPB)
                if it > 0 or s > 0:
                    prev = E[1 - s][:, 0:1]
                    if it == 0:
                        # gate both entry points of the it==0 chain
                        nc.vector.tensor_copy(prod[s][:, 0:1], prev)
                        nc.vector.tensor_copy(u1[s][:, 0, 0:1], prev)
                    else:
                        nc.vector.tensor_copy(X[s][:, 0, 0:1], prev)
                if it == 0:
                    tbt, sgt = tb0[s], sg0[s]
                else:
                    tbt, sgt = tb[s][:], sgn[s][:]
                    Xf = X[s][:].rearrange("p a b -> p (a b)")
                    Xu = X[s][:].bitcast(U16)
                    tbu = tb[s][:].bitcast(U16).rearrange("p a b -> p (a b)")
                    # X = gather(zrep) - E  (E pad lanes are kept 0, so the
                    # pad X stays at the BIG sentinel from the gather)
                    nc.vector.tensor_tensor(Xf, Xg[s][:, 0:EPB],
                                            E[s][:, 0:EPB], op=ALU.subtract)
                    # tb = |X| (exact; exclusion is positional, no tie-break)
                    nc.vector.tensor_single_scalar(
                        tbu, Xu.rearrange("p a b -> p (a b)"), 0x7FFF,
                        op=ALU.bitwise_and)
                    # sgn = +-0.5, never 0
                    nc.vector.tensor_scalar(sgt, X[s][:], 0.0, 0.5,
                                            op0=ALU.is_ge, op1=ALU.subtract)
                # parity: prod of +-0.5 -> pr = +-2a
                nc.vector.tensor_reduce(prod[s][:], sgt, axis=AX.X,
                                        op=ALU.mult)
                nc.vector.tensor_scalar(pr[s][:], prod[s][:], 0.0, 4.0 * al,
                                        op0=ALU.is_ge, op1=ALU.mult)
                nc.vector.tensor_single_scalar(pr[s][:], pr[s][:], 2.0 * al,
                                               op=ALU.subtract)
                # sp = sgn * pr (per-edge +-alpha signed scale; also serves
                # as a scheduler spacer in the tree's semaphore gaps)
                prb = pr[s][:].unsqueeze(2).broadcast_to([P, RPB, KPAD])
                nc.vector.tensor_tensor(sp[s][:], sgt, prb, op=ALU.mult)
                # exclusive-min tournament: em[k] = min over the row's other
                # 15 slots, computed from the half-split min tree + sibling
                # subtree views.  Position-based exclusion reproduces the
                # reference's tie semantics exactly (tied minima all see the
                # other tie), with full fp16 precision.
                nc.vector.tensor_tensor(u1[s][:], tbt[:, :, 0:8],
                                        tbt[:, :, 8:16], op=ALU.min)
                nc.vector.tensor_tensor(u2[s][:], u1[s][:, :, 0:4],
                                        u1[s][:, :, 4:8], op=ALU.min)
                nc.vector.tensor_tensor(u3[s][:], u2[s][:, :, 0:2],
                                        u2[s][:, :, 2:4], op=ALU.min)
                # m_a[k] = min(sibling leaf x[k^8], sibling pair u1[(k%8)^4])
                # (split over bit3 to stay within 3 free AP dims)
                x24 = tbt.rearrange("p a (h q l) -> p a h q l", h=2, q=2)
                u1f = u1[s][:].rearrange(
                    "p a (q l) -> p a q l", q=2)[:, :, ::-1, :]
                mav = ma[s][:].rearrange("p (a h q l) -> p a h q l",
                                         a=RPB, h=2, q=2)
                nc.vector.tensor_tensor(mav[:, :, 0], x24[:, :, 1], u1f,
                                        op=ALU.min)
                nc.vector.tensor_tensor(mav[:, :, 1], x24[:, :, 0], u1f,
                                        op=ALU.min)
                # m_b4[(b1,b0)] = min(u2[(b1^1,b0)], u3[b0^1]); em tiles it x4
                u2f = u2[s][:].rearrange(
                    "p a (q l) -> p a q l", q=2)[:, :, ::-1, :]
                u3f = u3[s][:][:, :, ::-1].unsqueeze(2).broadcast_to(
                    [P, RPB, 2, 2])
                mb4v = mb[s][:, 0:4 * RPB].rearrange(
                    "p (a q l) -> p a q l", a=RPB, q=2)
                nc.vector.tensor_tensor(mb4v, u2f, u3f, op=ALU.min)
                mb4t = mb[s][:, 0:4 * RPB].rearrange(
                    "p (a l) -> p a l", a=RPB).unsqueeze(2).broadcast_to(
                    [P, RPB, 4, 4])
                emv = em[s][:].rearrange("p (a h l) -> p a h l", a=RPB, h=4)
                mav2 = ma[s][:].rearrange("p (a h l) -> p a h l", a=RPB, h=4)
                nc.vector.tensor_tensor(emv, mav2, mb4t, op=ALU.min)
                # wE = max(em - beta, 0);  E = wE * sp  (pad lanes untouched)
                nc.vector.tensor_scalar(wE[s][:], em[s][:], float(beta[it]),
                                        0.0, op0=ALU.subtract, op1=ALU.max)
                wEv = wE[s][:].rearrange("p (a b) -> p a b", a=RPB)
                nc.vector.tensor_tensor(Eedge[:, :, 0:ROW_DEG],
                                        wEv[:, :, 0:ROW_DEG],
                                        sp[s][:, :, 0:ROW_DEG], op=ALU.mult)
                PHASE_LOG[(it, s)] = list(nc.inst_map)[-1]
                for i, (q, extra) in enumerate(aux):
                    nc.vector.tensor_tensor(
                        E[s][:, AUXB + i:AUXB + i + 1], E[s][:, q:q + 1],
                        E[s][:, extra:extra + 1], op=ALU.add)

            def strips_mm(s):
                """Strip gathers + accumulating one-hot matmuls for stream s."""
                for h in range(2):
                    o = D_KEEP * HALF * h
                    nc.gpsimd.indirect_copy(
                        G[s][:, h].rearrange("p d c -> p (d c)"),
                        E[s][:, 0:extw],
                        sidx[:, o // 16:(o + D_KEEP * HALF) // 16], True)
                    zps = zpsA[s] if h == 0 else zpsB[s]
                    for d in range(D_KEEP):
                        nc.tensor.matmul(zps[:], lhsT=wmat[:],
                                         rhs=G[s][:, h, d, :],
                                         start=(d == 0),
                                         stop=(d == D_KEEP - 1))

            def evac(s, it, on_pool=False):
                """PSUM -> zrep (iters 0,1) or zout + output DMA (final).
                on_pool routes the copy to GPSIMD: slower (495 vs 425ns) but
                avoids head-of-line blocking chain ACT ops behind an evac
                that waits on matmuls, and sits right before xgather in the
                Pool queue."""
                for h in range(2):
                    sl = slice(h * HALF, (h + 1) * HALF)
                    zps = zpsA[s] if h == 0 else zpsB[s]
                    if it == ITERS - 1:
                        # final evac: stream B's second half goes on the
                        # now-idle DVE so both halves copy concurrently
                        # (stream A's must stay on ACT: a DVE copy waiting
                        # on A's matmuls would head-of-line block B's chain)
                        if h == 0 or s == 0:
                            nc.scalar.activation(zout[s][:, sl], zps[:],
                                                 func=AF.Copy)
                        else:
                            nc.vector.tensor_copy(zout[s][:, sl], zps[:])
                        nc.sync.dma_start(out_d[s * B:(s + 1) * B, sl],
                                          zout[s][0:B, sl])
                    elif on_pool:
                        nc.gpsimd.tensor_copy(zrep[s][:, sl], zps[:])
                    else:
                        nc.scalar.activation(zrep[s][:, sl], zps[:],
                                             func=AF.Copy)

            def xgather(s):
                """Edge re-expansion for the next iteration's chain."""
                nc.gpsimd.indirect_copy(Xg[s][:], zrep[s][:, 0:N + 1],
                                        zidx[:], True)

            # Manual phase gates (us): without them the greedy Tile list
            # scheduler interleaves the two streams' DVE chains, which makes
            # both streams' E finish together and kills the chain/colsum
            # overlap.  Gating stream B's chain at stream A's chain end keeps
            # the pipeline: DVE runs A then B while A's colsum proceeds on
            # Pool/PE/ACT.  Times tuned against the deterministic timeline
            # simulator; they are scheduler hints (ordering), data deps still
            # guarantee correctness.
            for it in range(ITERS):
                chain(0, it)
                chain(1, it)
                for s in range(NSTR):
                    strips_mm(s)
                    evac(s, it)
                    if it < ITERS - 1:
                        xgather(s)

    if STRIP_WAITS:
        strip_same_engine_waits(nc)
    hoist_waits(nc)
    return nc


# ------------------------------------------------------------ host driver ----
_CACHE = {}


def kernel(r, H, alpha, beta):
    r = np.asarray(r, dtype=np.float32)
    H = np.asarray(H, dtype=np.float32)
    alpha_l = [float(x) for x in np.asarray(alpha).reshape(-1)]
    beta_l = [float(x) for x in np.asarray(beta).reshape(-1)]

    key = (H.tobytes(), tuple(alpha_l), tuple(beta_l))
    if key not in _CACHE:
        tables = build_tables(H)
        nc = build_bass(alpha_l, beta_l, tables["aux"])
        _CACHE[key] = (tables, nc)
    tables, nc = _CACHE[key]

    from concourse.bass_utils import run_bass_kernel_spmd
    stat = build_stat(tables)
    in_maps = []
    for c in range(8):
        rs = np.ascontiguousarray(r[c * BC:(c + 1) * BC])
        m = {"stat": stat,
             "r4": build_r4(rs, tables["rcols"]).reshape(P, NSTR * (1 + RW))}
        for s in range(NSTR):
            rstr = np.ascontiguousarray(rs[s * B:(s + 1) * B])
            m[f"ts0_{s}"] = build_ts0(
                rstr, tables["colidx"]).reshape(P, 2 * EPB)
        in_maps.append(m)
    # the first execution on a freshly-attached device occasionally fails
    # with NRT_EXEC_UNIT_UNRECOVERABLE; a retry succeeds
    last = None
    for _attempt in range(3):
        try:
            res = run_bass_kernel_spmd(nc, in_maps, core_ids=list(range(8)))
            break
        except Exception as e:  # noqa: BLE001
            last = e
    else:
        raise last
    out = np.concatenate([res.results[c]["out"] for c in range(8)], axis=0)
    return out.astype(np.float32)
